# revision 24
# baseline (speedup 1.0000x reference)
"""Ensemble MLP surrogate (16 models, 32->64->64->64->8, relu) on 8 TRN2 cores.

Strategy (data-parallel over batch, weights replicated):
  host packs x transposed + 4x row-replicated [128, B/8] per core in fp16;
  feature-on-partition layout with batch streaming as the matmul moving
  operand.  fp16 operands run the PE at 1 cycle/row with fast weight loads
  and allow tile_position row+col packing, so L1 runs as per-pair quads
  (4 concurrent matmuls) and L2/L3 as pair-duo quads; L4 packs 4 pairs per
  PSUM bank via column offsets.  Ensemble mean / sum-of-squares reductions
  run on the PE via selector matmuls; bias+ReLU epilogues (PSUM->SBUF) are
  load-balanced across the Vector and Scalar engines.
"""

import numpy as np

N_MODELS = 16
IN_DIM = 32
HID = 64
OUT_DIM = 8
BATCH = 131072
N_CORES = 8
B_CORE = BATCH // N_CORES  # 16384
TILE = 512  # matmul moving-operand columns (fp32 PSUM bank limit on out)
DTILE = 2 * TILE  # batch elements per pipeline step ("double tile")
NPAIR = N_MODELS // 2

# wpackr free-dim layout (fp16 matmul operands, 128 partitions)
# W1: pair blockdiag [64, 128] (model a rows 0-31 -> cols 0-63, model b rows
#     32-63 -> cols 64-127), replicated at rows 64-127 for the h=1 matmul.
# W2/W3: pair blockdiag [128, 128] (a: rows 0-63 -> cols 0-63, b: 64-127).
# One full-width matmul per (pair, half) replaces the old 2-matmul quads.
OFF_W1 = 0  # [128, 8, 128]
OFF_W2 = OFF_W1 + NPAIR * 128
OFF_W3 = OFF_W2 + NPAIR * 128
OFF_W4 = OFF_W3 + NPAIR * 128  # [128, 8, 32] pair blockdiag (cols 16-31 zero)
OFF_SELM = OFF_W4 + NPAIR * 32  # [128, 8]  mean selector (1/16)
OFF_SELS = OFF_SELM + 8  # [128, 8]  sumsq selector (1/16; 16/15 in sqrt)
WR = OFF_SELS + 8
# wpackb free-dim layout (fp32 biases)
OFF_B1 = 0  # [128, 8]
OFF_B2 = OFF_B1 + NPAIR  # [128, 8]
OFF_B3 = OFF_B2 + NPAIR  # [128, 8]
OFF_B4 = OFF_B3 + NPAIR  # [128, 2] (per L4 bank-group)
WB = OFF_B4 + 2


# estimated epilogue op costs (ns) for greedy DVE/ACT load balancing
def _act_cost(fd, psum_src=True):
    return ((172 if psum_src else 224) + fd) / 1.2


def _dve_cost(fd, psum_src=True, accel=1):
    return ((120 if psum_src else 58) + fd / accel) / 0.96


def pack_inputs(x, W1, b1, W2, b2, W3, b3, W4, b4, b_core=B_CORE, n_cores=N_CORES):
    """Host-side packing. Returns (xt_per_core list, wpackr fp16, wpackb f32)."""
    f32 = np.float32
    x = np.ascontiguousarray(x, dtype=f32)
    wpack = np.zeros((128, WR), f32)
    wpackb = np.zeros((128, WB), f32)

    w1v = wpack[:, OFF_W1 : OFF_W1 + NPAIR * 128].reshape(128, NPAIR, 128)
    w2v = wpack[:, OFF_W2 : OFF_W2 + NPAIR * 128].reshape(128, NPAIR, 128)
    w3v = wpack[:, OFF_W3 : OFF_W3 + NPAIR * 128].reshape(128, NPAIR, 128)
    w4v = wpack[:, OFF_W4 : OFF_W4 + NPAIR * 32].reshape(128, NPAIR, 32)
    for j in range(NPAIR):
        a, b = 2 * j, 2 * j + 1
        for r in (0, 64):  # replica for the h=0 / h=1 matmul row-groups
            w1v[r : r + 32, j, 0:HID] = W1[a]
            w1v[r + 32 : r + 64, j, HID:128] = W1[b]
        w2v[0:HID, j, 0:HID] = W2[a]
        w2v[HID:128, j, HID:128] = W2[b]
        w3v[0:HID, j, 0:HID] = W3[a]
        w3v[HID:128, j, HID:128] = W3[b]
        w4v[0:HID, j, 0:OUT_DIM] = W4[a]
        w4v[HID:128, j, OUT_DIM : 2 * OUT_DIM] = W4[b]

    selm = wpack[:, OFF_SELM : OFF_SELM + 8]
    sels = wpack[:, OFF_SELS : OFF_SELS + 8]
    b4v = wpackb[:, OFF_B4 : OFF_B4 + 2]
    for q in range(4):  # pair-within-group
        for c in range(2):  # model-within-pair
            for o in range(OUT_DIM):
                p = 32 * q + 8 * c + o
                selm[p, o] = 1.0 / 16.0
                sels[p, o] = 1.0 / 16.0  # exact in fp16; 16/15 applied at sqrt
                b4v[p, 0] = b4[2 * q + c, o]  # group A: pairs 0-3
                b4v[p, 1] = b4[2 * (q + 4) + c, o]  # group B: pairs 4-7
    for j in range(NPAIR):
        a, b = 2 * j, 2 * j + 1
        wpackb[0:HID, OFF_B1 + j] = b1[a]
        wpackb[HID:128, OFF_B1 + j] = b1[b]
        wpackb[0:HID, OFF_B2 + j] = b2[a]
        wpackb[HID:128, OFF_B2 + j] = b2[b]
        wpackb[0:HID, OFF_B3 + j] = b3[a]
        wpackb[HID:128, OFF_B3 + j] = b3[b]

    wpack16 = wpack.astype(np.float16)
    x16 = x.astype(np.float16)
    xt_per_core = []
    for c in range(n_cores):
        shard = x16[c * b_core : (c + 1) * b_core]  # [b_core, 32]
        xt = np.ascontiguousarray(np.tile(shard.T, (4, 1)))  # [128, b_core]
        xt_per_core.append(xt)
    return xt_per_core, wpack16, wpackb


def _emit(tc, ctx, xt, wr, wb, meant, stdt, b_core):
    import concourse.bass as bass  # noqa: F401
    from concourse import mybir

    nc = tc.nc
    f32 = mybir.dt.float32
    f16 = mybir.dt.float16
    AF = mybir.ActivationFunctionType
    ALU = mybir.AluOpType

    n_dt = b_core // DTILE

    consts = ctx.enter_context(tc.tile_pool(name="consts", bufs=1))
    xp = ctx.enter_context(tc.tile_pool(name="xp", bufs=4))
    hp = [
        ctx.enter_context(tc.tile_pool(name=f"h{i}p", bufs=b))
        for i, b in enumerate((6, 6, 10))
    ]
    prp = ctx.enter_context(tc.tile_pool(name="prp", bufs=6))
    sqp = ctx.enter_context(tc.tile_pool(name="sqp", bufs=6))
    smp = ctx.enter_context(tc.tile_pool(name="smp", bufs=4))  # small stats sbuf
    outp = ctx.enter_context(tc.tile_pool(name="outp", bufs=4))
    # PSUM budget (8 banks): 1-bank [128,512] tiles, bufs=2 per stage tag
    # (ph1 2 + ph2 2 + ph34/p4 2 + pst 2).  Two slots per stage keep two
    # (pair, half) chains in flight per layer, so slot reuse only waits on
    # the same stage's other chain and ACT+DVE always see ready epilogues.
    php = ctx.enter_context(tc.tile_pool(name="php", bufs=2, space="PSUM"))
    pstp = ctx.enter_context(tc.tile_pool(name="pstp", bufs=2, space="PSUM"))

    cw = consts.tile([128, WR], f16)
    # split the ~850KB weight pack across 4 DMA queues, W1 first so the
    # first L1 matmuls only wait for their own chunk
    cwb = consts.tile([128, WB], f32)
    nc.gpsimd.dma_start(out=cwb, in_=wb)
    nc.gpsimd.dma_start(
        out=cw[:, OFF_W1 : OFF_W1 + NPAIR * 128],
        in_=wr[:, OFF_W1 : OFF_W1 + NPAIR * 128],
    )
    nc.sync.dma_start(
        out=cw[:, OFF_W2 : OFF_W2 + NPAIR * 128],
        in_=wr[:, OFF_W2 : OFF_W2 + NPAIR * 128],
    )
    nc.scalar.dma_start(
        out=cw[:, OFF_W3 : OFF_W3 + NPAIR * 128],
        in_=wr[:, OFF_W3 : OFF_W3 + NPAIR * 128],
    )
    nc.scalar.dma_start(out=cw[:, OFF_W4:WR], in_=wr[:, OFF_W4:WR])
    w1v = cw[:, OFF_W1 : OFF_W1 + NPAIR * 128].rearrange("p (j f) -> p j f", f=128)
    w2v = cw[:, OFF_W2 : OFF_W2 + NPAIR * 128].rearrange("p (j f) -> p j f", f=128)
    w3v = cw[:, OFF_W3 : OFF_W3 + NPAIR * 128].rearrange("p (j f) -> p j f", f=128)
    w4v = cw[:, OFF_W4 : OFF_W4 + NPAIR * 32].rearrange("p (j f) -> p j f", f=32)
    selm = cw[:, OFF_SELM : OFF_SELM + 8]
    sels = cw[:, OFF_SELS : OFF_SELS + 8]

    # ACT table warmup: touch Sqrt and Square at t=0 so both table-set
    # loads (~2.7us each) overlap the initial weight DMA instead of
    # stalling the scalar engine mid-stream at first use
    warm = smp.tile([1, 2], f32, tag="warm")
    nc.gpsimd.memset(warm, 1.0)
    nc.scalar.activation(warm, warm, mybir.ActivationFunctionType.Sqrt)
    nc.scalar.activation(warm, warm, mybir.ActivationFunctionType.Square)

    # greedy engine balancer for PSUM->SBUF epilogues + SBUF squares
    eng_ns = {"act": 0.0, "dve": 0.0}

    def epilogue(out, in_, bias, relu):
        fd = out.free_size()
        if eng_ns["act"] + _act_cost(fd) <= eng_ns["dve"] + _dve_cost(fd):
            eng_ns["act"] += _act_cost(fd)
            nc.scalar.activation(
                out, in_, AF.Relu if relu else AF.Identity, bias=bias, scale=1.0
            )
        else:
            eng_ns["dve"] += _dve_cost(fd)
            if relu:
                nc.vector.tensor_scalar(
                    out, in_, bias, 0.0, op0=ALU.add, op1=ALU.max
                )
            else:
                nc.vector.tensor_scalar(out, in_, bias, None, op0=ALU.add)

    def sbuf_square(out, in_):
        # SBUF-only elementwise: run on GPSIMD, the idle engine (ACT+DVE are
        # the bottleneck; Q7 at 0.42 eff is still free capacity)
        nc.gpsimd.tensor_mul(out, in_, in_)

    pending = []

    def emit_stats(x0, pr, h):
        mean_ps = pstp.tile([8, TILE], f32, tag="pst", name="mean_ps")
        sq_ps = pstp.tile([8, TILE], f32, tag="pst", name="sq_ps")
        # alternate PSUM banks between consecutive MMs so they pipeline
        nc.tensor.matmul(
            out=mean_ps, lhsT=selm, rhs=pr[(0, h)][0], start=True, stop=False
        )
        nc.tensor.matmul(
            out=sq_ps, lhsT=sels, rhs=pr[(0, h)][1], start=True, stop=False
        )
        nc.tensor.matmul(
            out=mean_ps, lhsT=selm, rhs=pr[(1, h)][0], start=False, stop=True
        )
        nc.tensor.matmul(
            out=sq_ps, lhsT=sels, rhs=pr[(1, h)][1], start=False, stop=True
        )

        mean_sb = outp.tile([8, TILE], f32, tag="mean")
        if eng_ns["act"] + _act_cost(TILE) <= eng_ns["dve"] + _dve_cost(TILE):
            eng_ns["act"] += _act_cost(TILE)
            nc.scalar.copy(out=mean_sb, in_=mean_ps)
        else:
            eng_ns["dve"] += _dve_cost(TILE)
            nc.vector.tensor_copy(mean_sb, mean_ps)
        nc.sync.dma_start(
            out=meant[:, x0 + h * TILE : x0 + (h + 1) * TILE], in_=mean_sb
        )
        m2 = smp.tile([8, TILE], f32, tag="m2")
        # m2 = mean^2 from SBUF on GPSIMD (frees the mean PSUM bank after
        # the single ACT copy; keeps ACT/DVE for PSUM-crossing work)
        nc.gpsimd.tensor_mul(m2, mean_sb, mean_sb)
        nvar = smp.tile([8, TILE], f32, tag="nvar")
        # nvar = mean^2 - E[p^2] == -var * 15/16
        nc.vector.tensor_sub(nvar, m2, sq_ps)
        eng_ns["dve"] += _dve_cost(TILE)
        std_sb = outp.tile([8, TILE], f32, tag="std")
        nc.scalar.activation(
            out=std_sb, in_=nvar, func=AF.Sqrt, scale=-16.0 / 15.0
        )
        eng_ns["act"] += _act_cost(TILE, psum_src=False)
        nc.sync.dma_start(
            out=stdt[:, x0 + h * TILE : x0 + (h + 1) * TILE], in_=std_sb
        )

    for t in range(n_dt):
        x0 = t * DTILE
        xt_t = xp.tile([128, 2, TILE], f16, tag="xt")
        nc.sync.dma_start(
            out=xt_t,
            in_=xt[:, x0 : x0 + DTILE].rearrange("p (h n) -> p h n", n=TILE),
        )

        pr = {}
        h3s = {}

        for d in range(NPAIR // 2):
            j0, j1 = 2 * d, 2 * d + 1
            # L1 per (pair, half): 2 MMs on row-groups 2h/2h+1; the four
            # (j, h) tiles of a pair pack the array as the same 4-MM quad
            h1 = {}
            for j in (j0, j1):
                for h in range(2):
                    ph1 = php.tile([128, TILE], f32, tag="ph1", name=f"ph1_{j}_{h}")
                    nc.tensor.matmul(
                        out=ph1,
                        lhsT=w1v[64 * h : 64 * h + 64, j, :],
                        rhs=xt_t[64 * h : 64 * h + 64, h, :],
                        start=True, stop=True,
                        tile_position=(64 * h, 0),
                    )
                    h1[(j, h)] = hp[0].tile(
                        [128, TILE], f16, tag="h1", name=f"h1_{j}_{h}"
                    )
                    epilogue(
                        h1[(j, h)], ph1,
                        cwb[:, OFF_B1 + j : OFF_B1 + j + 1], relu=True,
                    )

            if d in (1, 2) and pending:  # prev dtile's stats, split in two
                x0p, prs = pending[0]
                emit_stats(x0p, prs, d - 1)
                if d == 2:
                    pending.pop(0)

            # L2 per (pair, half): 2 block MMs; odd pairs parity-swapped so
            # (j0, h) + (j1, h) tiles together cover the full array
            h2 = {}
            for j in (j0, j1):
                for h in range(2):
                    ph2 = php.tile([128, TILE], f32, tag="ph2", name=f"ph2_{j}_{h}")
                    nc.tensor.matmul(
                        out=ph2, lhsT=w2v[:, j, :], rhs=h1[(j, h)],
                        start=True, stop=True,
                    )
                    h2[(j, h)] = hp[1].tile(
                        [128, TILE], f16, tag="h2", name=f"h2_{j}_{h}"
                    )
                    epilogue(
                        h2[(j, h)], ph2,
                        cwb[:, OFF_B2 + j : OFF_B2 + j + 1], relu=True,
                    )

            # L3 per (pair, half): same quad; w3v un-swaps odd pairs
            for j in (j0, j1):
                for h in range(2):
                    ph3 = php.tile([128, TILE], f32, tag="ph34", name=f"ph3_{j}_{h}")
                    nc.tensor.matmul(
                        out=ph3, lhsT=w3v[:, j, :], rhs=h2[(j, h)],
                        start=True, stop=True,
                    )
                    h3s[(j, h)] = hp[2].tile(
                        [128, TILE], f16, tag="h3", name=f"h3_{j}_{h}"
                    )
                    epilogue(
                        h3s[(j, h)], ph3,
                        cwb[:, OFF_B3 + j : OFF_B3 + j + 1], relu=True,
                    )

            if d % 2 == 1:
                # L4 group g: per half, 4 blockdiag MMs (one per pair) at
                # distinct col-groups into a 1-bank tile riding the ph34 tag
                g = d // 2
                for h in range(2):
                    p4gh = php.tile([128, TILE], f32, tag="ph34", name=f"p4_{g}_{h}")
                    for j in range(4 * g, 4 * g + 4):
                        q = j % 4
                        nc.tensor.matmul(
                            out=p4gh[32 * q : 32 * q + 32, :],
                            lhsT=w4v[:, j, :],
                            rhs=h3s[(j, h)],
                            start=True, stop=True,
                            tile_position=(0, 32 * q),
                        )
                    prt = prp.tile([128, TILE], f16, tag="pr", name=f"pr_{g}_{h}")
                    epilogue(
                        prt, p4gh,
                        cwb[:, OFF_B4 + g : OFF_B4 + g + 1], relu=False,
                    )
                    sqt = sqp.tile([128, TILE], f16, tag="sq", name=f"sq_{g}_{h}")
                    sbuf_square(sqt, prt)
                    pr[(g, h)] = (prt, sqt)

        # ensemble stats are DEFERRED into the next double-tile's dense
        # region so the PE tail never idles long enough to re-throttle (HAM)
        pending.append((x0, pr))

    for x0p, prs in pending:  # drain remaining deferred stats
        emit_stats(x0p, prs, 0)
        emit_stats(x0p, prs, 1)


def build(b_core=B_CORE, num_devices=N_CORES, bias=True):
    from contextlib import ExitStack

    import concourse.bacc as bacc
    import concourse.hw_specs as hw_specs
    import concourse.tile as tile
    from concourse import mybir

    f32 = mybir.dt.float32
    f16 = mybir.dt.float16
    nc = bacc.Bacc(
        "TRN2", target_bir_lowering=False, debug=False, num_devices=num_devices
    )
    xt = nc.dram_tensor("xt", [128, b_core], f16, kind="ExternalInput").ap()
    wr = nc.dram_tensor("wpackr", [128, WR], f16, kind="ExternalInput").ap()
    wb = nc.dram_tensor("wpackb", [128, WB], f32, kind="ExternalInput").ap()
    meant = nc.dram_tensor("meant", [8, b_core], f32, kind="ExternalOutput").ap()
    stdt = nc.dram_tensor("stdt", [8, b_core], f32, kind="ExternalOutput").ap()
    # Bias the scheduler's cost model toward measured per-op times (matmuls
    # issue at ~388ns/512col on HW vs the 213ns model; ACT/DVE ~15% above
    # model) so the static engine order leaves realistic slack.  Restored
    # right after scheduling.
    spec = hw_specs.TRN2Spec
    saved = (spec.PE_CYCLE, spec.PE_CYCLE_PSTATE_MID, dict(spec.CYCLE_T))
    if bias:
        pass  # biasing measured worse on HW (455us vs 365us); disabled
    try:
        with tile.TileContext(nc) as tc:
            with ExitStack() as ctx:
                _emit(tc, ctx, xt, wr, wb, meant, stdt, b_core)
        nc.compile()
    finally:
        spec.PE_CYCLE, spec.PE_CYCLE_PSTATE_MID = saved[0], saved[1]
        spec.CYCLE_T = saved[2]
    return nc


_NC_CACHE = {}


def kernel(x, W1, b1, W2, b2, W3, b3, W4, b4):
    from concourse.bass_utils import run_bass_kernel_spmd

    key = ("full", B_CORE)
    if key not in _NC_CACHE:
        _NC_CACHE[key] = build(B_CORE, N_CORES)
    nc = _NC_CACHE[key]

    xt_per_core, wpackr, wpackb = pack_inputs(
        np.asarray(x), np.asarray(W1), np.asarray(b1), np.asarray(W2),
        np.asarray(b2), np.asarray(W3), np.asarray(b3), np.asarray(W4),
        np.asarray(b4),
    )
    in_maps = [
        {"xt": xt_per_core[c], "wpackr": wpackr, "wpackb": wpackb}
        for c in range(N_CORES)
    ]
    res = run_bass_kernel_spmd(nc, in_maps, list(range(N_CORES))).results
    mean = np.concatenate([res[c]["meant"] for c in range(N_CORES)], axis=1).T
    std = np.concatenate([res[c]["stdt"] for c in range(N_CORES)], axis=1).T
    return np.ascontiguousarray(mean), np.ascontiguousarray(std)

